# revision 18
# baseline (speedup 1.0000x reference)
"""Trainium2 Bass kernel for the soft-target loss:

    probs = softmax(outputs, axis=1)          # [B, C]
    p_t   = probs[i, targets[i]]              # [B]
    loss  = mean(2 - 2 * p_t)                 # scalar

Strategy (pure data parallel over 8 NeuronCores):
  - The f32 logits are cast to bf16 on the host before staging: the
    kernel is HBM-bound at f32 (65.5 MB/core) and individual cores
    intermittently lose HBM bandwidth to outside traffic (~320 GB/s
    floor observed).  bf16 halves the stream to 32.8 MB/core so even a
    starved core streams in ~100 us, below the compute floor.  End-to-
    end loss error from bf16 logits is ~1e-8 (tolerance is 2e-2): the
    per-row softmax ratio is scale-free and errors average out over
    131072 rows.
  - Rows are sorted by target class on the host (any row permutation is
    valid for a batch mean).  Each 128-row sub-tile then covers a ~2-3
    class range, so the target-logit gather only needs to scan a 64-wide
    class window instead of all 1000 columns.  The window base is
    (lo + OFF[j]): OFF is a static schedule (classes advance
    128/131.072 per sub-tile for a uniform target distribution) and lo
    is a per-core runtime register loaded from a tiny config input.
    The host verifies the schedule covers the actual targets and falls
    back to a full-scan f32 program if not (never happens for uniform
    targets).
  - Per 2 MB stream tile (4 rows/partition): ScalarE does one batched
    exp into a bf16 scratch; VectorE accumulates per-row sums from the
    scratch (bf16 single-source runs in the fast DVE mode) and does the
    64-wide one-hot gather per sub-tile.  Windows may read up to 64
    elements past a row into the next row (never matching the one-hot);
    a 64-element pad at the tile end is zeroed so the last row is safe.
  - Combine (p_t = exp(g) * 1/rowsum) runs in chunks mid-stream; the
    tail handles 4 columns plus a [128,1]x[128,1] matmul reduction.
  - Host sums the 8 partials: loss = 2 - 2 * total / B.
"""

import numpy as np

B, C = 131072, 1000
N_CORES = 8
P = 128                      # SBUF partitions
RPP = 4                      # rows per partition per mid-stream tile
NJ = 128                     # column-groups (128 rows each) per core
ROWS = P * NJ                # rows per core
W = 64                       # gather window width (classes)
# Static window schedule: classes advance 128/131.072 per column group.
OFF = [int(j * 128 * C / B) for j in range(NJ)]

_PROGRAMS = {}


def _tile_plan():
    return [(1, 2), (2, 1), (4, 30), (2, 1), (1, 2)]


def _iter_tiles():
    row, col = 0, 0
    for g_rpp, cnt in _tile_plan():
        for _ in range(cnt):
            yield row, col, g_rpp
            row += P * g_rpp
            col += g_rpp


def _build_sorted(ncols=C):
    """bf16 stream + windowed gather + batched exp (fast path)."""
    from contextlib import ExitStack

    import concourse.tile as tile
    from concourse import bacc, mybir

    nj = NJ

    nc = bacc.Bacc(
        "TRN2",
        target_bir_lowering=False,
        debug=False,
        enable_asserts=False,
        num_devices=N_CORES,
    )
    x = nc.dram_tensor(
        "x", [ROWS, ncols], mybir.dt.bfloat16, kind="ExternalInput"
    ).ap()
    tfr = nc.dram_tensor("tfr", [P, nj], mybir.dt.bfloat16, kind="ExternalInput").ap()
    cfg = nc.dram_tensor("cfg", [1, 1], mybir.dt.uint32, kind="ExternalInput").ap()
    out = nc.dram_tensor("partial", [1, 1], mybir.dt.float32, kind="ExternalOutput").ap()

    with tile.TileContext(nc) as tc, ExitStack() as ctx:
        stream = ctx.enter_context(tc.tile_pool(name="stream", bufs=3))
        psum = ctx.enter_context(tc.tile_pool(name="psum", bufs=2, space="PSUM"))
        persist = ctx.enter_context(tc.tile_pool(name="persist", bufs=1))
        lreg = ctx.enter_context(nc.vector.register(name="lreg"))

        sums = persist.tile([P, nj], mybir.dt.float32)
        g = persist.tile([P, nj], mybir.dt.float32)
        eg = persist.tile([P, nj], mybir.dt.float32)
        rec = persist.tile([P, nj], mybir.dt.float32)
        prod = persist.tile([P, nj], mybir.dt.float32)
        tf_t = persist.tile([P, nj], mybir.dt.bfloat16)
        cfg_t = persist.tile([1, 1], mybir.dt.uint32)
        # Small loads on the ACT HWDGE queue: keeps the sync queue free so
        # the first stream DMA issues immediately.
        nc.scalar.dma_start(cfg_t[:], cfg)
        nc.scalar.dma_start(tf_t[:], tfr)

        # Per-core window base register (vector engine: used in vector APs).
        nc.vector.load(lreg, cfg_t[:])
        lo = nc.vector.snap(lreg, min_val=0, max_val=C - W)

        warm = persist.tile([P, 1], mybir.dt.float32)
        nc.gpsimd.memset(warm[:], 0.0)
        nc.scalar.activation(warm[:], warm[:], mybir.ActivationFunctionType.Exp)

        # Window-relative class indices 0..W-1, replicated per partition.
        iota_i = persist.tile([P, W], mybir.dt.int32)
        nc.gpsimd.iota(iota_i[:], pattern=[[1, W]], base=0, channel_multiplier=0)
        iota_b = persist.tile([P, W], mybir.dt.bfloat16)
        nc.vector.tensor_copy(iota_b[:], iota_i[:])

        ones = persist.tile([P, 1], mybir.dt.float32)
        nc.vector.memset(ones[:], 1.0)
        zeros_b = persist.tile([P, ncols], mybir.dt.bfloat16)
        nc.vector.memset(zeros_b[:], 0.0)

        def combine(a, b):
            h = slice(a, b)
            nc.scalar.activation(eg[:, h], g[:, h], mybir.ActivationFunctionType.Exp)
            nc.vector.reciprocal(rec[:, h], sums[:, h])
            nc.vector.tensor_mul(prod[:, h], eg[:, h], rec[:, h])

        boundaries = [32, 64, 96, nj - 4]
        done = 0
        pad_done = {}

        from concourse.bass import ds

        for row0, col0, t_rpp in _iter_tiles():
            xt = x[row0 : row0 + P * t_rpp, :].rearrange("(p r) c -> p (r c)", p=P)
            n = t_rpp * ncols
            t = stream.tile(
                [P, n + 2 * W],
                mybir.dt.bfloat16,
                name=f"t{t_rpp}",
                tag=f"t{t_rpp}",
                bufs=3,
            )
            nc.sync.dma_start(t[:, 0:n], xt)
            # Zero the tail pad so last-row windows read finite bf16.  The
            # pool rotates 3 buffers per tag and nothing else writes the pad
            # bytes, so only the first rotation needs the memset.
            if pad_done.get(t_rpp, 0) < 3:
                nc.vector.memset(t[:, n : n + 2 * W], 0.0)
                pad_done[t_rpp] = pad_done.get(t_rpp, 0) + 1
            # First ns rows: exp + per-row sum on ScalarE (accumulator).
            # Mid tiles mostly keep one (a few keep zero): with segmented
            # DVE reduces the engines balance at ~34 scalar-summed columns
            # (each shifted column is +0.59us ScalarE / -1.04us VectorE).
            if t_rpp == 4:
                mid_idx = (col0 - 4) // 4
                ns = 0 if mid_idx % 8 == 3 else 1
            else:
                ns = min(t_rpp, 2)
            for r in range(ns):
                j = col0 + r
                scr0 = psum.tile([P, ncols], mybir.dt.float32, name="scr0")
                nc.scalar.activation(
                    scr0[:],
                    t[:, r * ncols : (r + 1) * ncols],
                    mybir.ActivationFunctionType.Exp,
                    accum_out=sums[:, j : j + 1],
                )
            # Remaining rows: one batched exp into a bf16 scratch; all their
            # row sums via ONE segmented DVE reduce (3D access pattern).
            if t_rpp > ns:
                scr = stream.tile(
                    [P, (t_rpp - ns) * ncols],
                    mybir.dt.bfloat16,
                    name=f"s{t_rpp}_{ns}",
                    tag=f"s{t_rpp}_{ns}",
                    bufs=2,
                )
                nc.scalar.activation(
                    scr[:], t[:, ns * ncols : n], mybir.ActivationFunctionType.Exp
                )
                nc.vector.tensor_reduce(
                    sums[:, col0 + ns : col0 + t_rpp],
                    scr[:].rearrange("p (k c) -> p k c", c=ncols),
                    axis=mybir.AxisListType.X,
                    op=mybir.AluOpType.add,
                )
            # Windowed one-hot gathers (no accumulator), then one segmented
            # reduce collapses all windows of the tile into g.
            msk4 = stream.tile(
                [P, t_rpp * W],
                mybir.dt.bfloat16,
                name=f"m{t_rpp}",
                tag=f"m{t_rpp}",
                bufs=2,
            )
            for r in range(t_rpp):
                j = col0 + r
                nc.vector.scalar_tensor_tensor(
                    out=msk4[:, r * W : (r + 1) * W],
                    in0=iota_b[:],
                    scalar=tf_t[:, j : j + 1],
                    in1=t[:, ds(lo + (r * ncols + OFF[j]), W)],
                    op0=mybir.AluOpType.is_equal,
                    op1=mybir.AluOpType.mult,
                )
            nc.vector.tensor_reduce(
                g[:, col0 : col0 + t_rpp],
                msk4[:].rearrange("p (k w) -> p k w", w=W),
                axis=mybir.AxisListType.X,
                op=mybir.AluOpType.add,
            )
            while boundaries and col0 + t_rpp >= boundaries[0]:
                combine(done, boundaries[0])
                done = boundaries.pop(0)

        combine(done, nj)
        pt = persist.tile([P, 1], mybir.dt.float32)
        nc.vector.tensor_reduce(
            pt[:], prod[:], axis=mybir.AxisListType.X, op=mybir.AluOpType.add
        )
        acc = psum.tile([1, 1], mybir.dt.float32, name="acc", bufs=1)
        nc.tensor.matmul(acc[:], lhsT=pt[:], rhs=ones[:], start=True, stop=True)
        res = persist.tile([1, 1], mybir.dt.float32)
        nc.vector.tensor_copy(res[:], acc[:])
        nc.sync.dma_start(out, res[:])

    nc.compile()
    return nc


def _build_fullscan(ncols=C):
    """f32 full-scan fallback (correct for any targets)."""
    from contextlib import ExitStack

    import concourse.tile as tile
    from concourse import bacc, mybir

    nj = NJ

    nc = bacc.Bacc(
        "TRN2",
        target_bir_lowering=False,
        debug=False,
        enable_asserts=False,
        num_devices=N_CORES,
    )
    x = nc.dram_tensor("x", [ROWS, ncols], mybir.dt.float32, kind="ExternalInput").ap()
    tf = nc.dram_tensor("tf", [P, nj], mybir.dt.float32, kind="ExternalInput").ap()
    out = nc.dram_tensor("partial", [1, 1], mybir.dt.float32, kind="ExternalOutput").ap()

    with tile.TileContext(nc) as tc, ExitStack() as ctx:
        stream = ctx.enter_context(tc.tile_pool(name="stream", bufs=3))
        psum = ctx.enter_context(tc.tile_pool(name="psum", bufs=2, space="PSUM"))
        persist = ctx.enter_context(tc.tile_pool(name="persist", bufs=1))

        sums = persist.tile([P, nj], mybir.dt.float32)
        g = persist.tile([P, nj], mybir.dt.float32)
        eg = persist.tile([P, nj], mybir.dt.float32)
        rec = persist.tile([P, nj], mybir.dt.float32)
        prod = persist.tile([P, nj], mybir.dt.float32)
        tf_t = persist.tile([P, nj], mybir.dt.float32)
        nc.scalar.dma_start(tf_t[:], tf)

        warm = persist.tile([P, 1], mybir.dt.float32)
        nc.gpsimd.memset(warm[:], 0.0)
        nc.scalar.activation(warm[:], warm[:], mybir.ActivationFunctionType.Exp)

        iota_i = persist.tile([P, ncols], mybir.dt.int32)
        nc.gpsimd.iota(iota_i[:], pattern=[[1, ncols]], base=0, channel_multiplier=0)
        iota_f = persist.tile([P, ncols], mybir.dt.float32)
        nc.vector.tensor_copy(iota_f[:], iota_i[:])

        ones = persist.tile([P, 1], mybir.dt.float32)
        nc.vector.memset(ones[:], 1.0)

        def combine(a, b):
            h = slice(a, b)
            nc.scalar.activation(eg[:, h], g[:, h], mybir.ActivationFunctionType.Exp)
            nc.vector.reciprocal(rec[:, h], sums[:, h])
            nc.vector.tensor_mul(prod[:, h], eg[:, h], rec[:, h])

        boundaries = [32, 64, 96, nj - 4]
        done = 0

        for row0, col0, t_rpp in _iter_tiles():
            xt = x[row0 : row0 + P * t_rpp, :].rearrange("(p r) c -> p (r c)", p=P)
            t = stream.tile(
                [P, t_rpp * ncols],
                mybir.dt.float32,
                name=f"t{t_rpp}",
                tag=f"t{t_rpp}",
                bufs=3,
            )
            nc.sync.dma_start(t[:], xt)
            for r in range(t_rpp):
                j = col0 + r
                xs = t[:, r * ncols : (r + 1) * ncols]
                scr = psum.tile([P, ncols], mybir.dt.float32, name="scr")
                nc.scalar.activation(
                    scr[:],
                    xs,
                    mybir.ActivationFunctionType.Exp,
                    accum_out=sums[:, j : j + 1],
                )
                msk = stream.tile([P, ncols], mybir.dt.float32, name="msk", bufs=2)
                nc.vector.scalar_tensor_tensor(
                    out=msk[:],
                    in0=iota_f[:],
                    scalar=tf_t[:, j : j + 1],
                    in1=xs,
                    op0=mybir.AluOpType.is_equal,
                    op1=mybir.AluOpType.mult,
                    accum_out=g[:, j : j + 1],
                )
            while boundaries and col0 + t_rpp >= boundaries[0]:
                combine(done, boundaries[0])
                done = boundaries.pop(0)

        combine(done, nj)
        pt = persist.tile([P, 1], mybir.dt.float32)
        nc.vector.tensor_reduce(
            pt[:], prod[:], axis=mybir.AxisListType.X, op=mybir.AluOpType.add
        )
        acc = psum.tile([1, 1], mybir.dt.float32, name="acc", bufs=1)
        nc.tensor.matmul(acc[:], lhsT=pt[:], rhs=ones[:], start=True, stop=True)
        res = persist.tile([1, 1], mybir.dt.float32)
        nc.vector.tensor_copy(res[:], acc[:])
        nc.sync.dma_start(out, res[:])

    nc.compile()
    return nc


def _dev_perm():
    """idx_dev[row0 + p*rpp + r] = (col0 + r) * 128 + p  (per-core local)."""
    idx = np.empty(ROWS, dtype=np.int64)
    for row0, col0, t_rpp in _iter_tiles():
        p = np.arange(P)[:, None]
        r = np.arange(t_rpp)[None, :]
        idx[(row0 + p * t_rpp + r).ravel()] = ((col0 + r) * P + p).ravel()
    return idx


def _plan_windows(tsc):
    """Given a core's ascending-sorted targets, pick the runtime window
    base lo such that [lo+OFF[j], lo+OFF[j]+W) covers sub-tile j's
    targets for all j. Returns lo or None if infeasible."""
    mint = tsc.reshape(NJ, P)[:, 0].astype(np.int64)
    maxt = tsc.reshape(NJ, P)[:, -1].astype(np.int64)
    off = np.asarray(OFF, dtype=np.int64)
    lo_low = int(np.max(maxt - (W - 1) - off))
    lo_high = int(np.min(mint - off))
    if lo_low > lo_high:
        return None
    lo = (lo_low + lo_high) // 2
    return int(np.clip(lo, 0, C - W))


def _run(outputs, targets, trace=False):
    import ml_dtypes

    from concourse import bass_utils

    outputs = np.ascontiguousarray(np.asarray(outputs, dtype=np.float32))
    targets = np.asarray(targets).astype(np.int64)

    # Sort rows by target; shard contiguous sorted ranges per core.
    order = np.argsort(targets, kind="stable")
    dev = _dev_perm()
    plans = []
    ok = True
    for i in range(N_CORES):
        sl = order[i * ROWS : (i + 1) * ROWS]
        tsc = targets[sl]
        lo = _plan_windows(tsc)
        if lo is None:
            ok = False
            break
        plans.append((sl, tsc, lo))

    in_maps = []
    if ok:
        key = "sorted"
        if key not in _PROGRAMS:
            _PROGRAMS[key] = _build_sorted()
        prog = _PROGRAMS[key]
        x16 = outputs.astype(ml_dtypes.bfloat16)
        off = np.asarray(OFF, dtype=np.int64)
        for sl, tsc, lo in plans:
            xd = x16[sl[dev]]
            # tf_rel[p, j] = t - (lo + OFF[j])  in [0, W)
            rel = (tsc.reshape(NJ, P).T - (lo + off)[None, :]).astype(np.float32)
            assert rel.min() >= 0 and rel.max() < W
            in_maps.append(
                {
                    "x": np.ascontiguousarray(xd),
                    "tfr": rel.astype(ml_dtypes.bfloat16),
                    "cfg": np.array([[lo]], dtype=np.uint32),
                }
            )
    else:
        key = "fullscan"
        if key not in _PROGRAMS:
            _PROGRAMS[key] = _build_fullscan()
        prog = _PROGRAMS[key]
        for i in range(N_CORES):
            sl = slice(i * ROWS, (i + 1) * ROWS)
            tfv = np.empty((P, NJ), dtype=np.float32)
            tshard = targets[sl].astype(np.float32)
            for row0, col0, t_rpp in _iter_tiles():
                ridx = (
                    row0
                    + np.arange(P)[:, None] * t_rpp
                    + np.arange(t_rpp)[None, :]
                )
                tfv[:, col0 : col0 + t_rpp] = tshard[ridx]
            in_maps.append({"x": outputs[sl], "tf": tfv})

    kw = {"trace_cores": list(range(N_CORES))} if trace else {}
    results = bass_utils.run_bass_kernel_spmd(
        prog, in_maps, core_ids=list(range(N_CORES)), trace=trace, **kw
    )
    total = sum(float(r["partial"][0, 0]) for r in results.results)
    loss = np.float32(2.0) - np.float32(2.0) * np.float32(total / B)
    return np.asarray(loss, dtype=np.float32), results


def kernel(outputs, targets):
    loss, _ = _run(outputs, targets, trace=False)
    return loss


# revision 19
# speedup vs baseline: 1.1266x; 1.1266x over previous
"""Trainium2 Bass kernel for the soft-target loss:

    probs = softmax(outputs, axis=1)          # [B, C]
    p_t   = probs[i, targets[i]]              # [B]
    loss  = mean(2 - 2 * p_t)                 # scalar

Strategy (pure data parallel over 8 NeuronCores):
  - The f32 logits are cast to bf16 on the host before staging: the
    kernel is HBM-bound at f32 (65.5 MB/core) and individual cores
    intermittently lose HBM bandwidth to outside traffic (~320 GB/s
    floor observed).  bf16 halves the stream to 32.8 MB/core so even a
    starved core streams in ~100 us, below the compute floor.  End-to-
    end loss error from bf16 logits is ~1e-8 (tolerance is 2e-2): the
    per-row softmax ratio is scale-free and errors average out over
    131072 rows.
  - Rows are sorted by target class on the host (any row permutation is
    valid for a batch mean).  Each 128-row sub-tile then covers a ~2-3
    class range, so the target-logit gather only needs to scan a 64-wide
    class window instead of all 1000 columns.  The window base is
    (lo + OFF[j]): OFF is a static schedule (classes advance
    128/131.072 per sub-tile for a uniform target distribution) and lo
    is a per-core runtime register loaded from a tiny config input.
    The host verifies the schedule covers the actual targets and falls
    back to a full-scan f32 program if not (never happens for uniform
    targets).
  - Per 2 MB stream tile (4 rows/partition): ScalarE does one batched
    exp into a bf16 scratch; VectorE accumulates per-row sums from the
    scratch (bf16 single-source runs in the fast DVE mode) and does the
    64-wide one-hot gather per sub-tile.  Windows may read up to 64
    elements past a row into the next row (never matching the one-hot);
    a 64-element pad at the tile end is zeroed so the last row is safe.
  - Combine (p_t = exp(g) * 1/rowsum) runs in chunks mid-stream; the
    tail handles 4 columns plus a [128,1]x[128,1] matmul reduction.
  - Host sums the 8 partials: loss = 2 - 2 * total / B.
"""

import numpy as np

B, C = 131072, 1000
N_CORES = 8
P = 128                      # SBUF partitions
RPP = 4                      # rows per partition per mid-stream tile
NJ = 128                     # column-groups (128 rows each) per core
ROWS = P * NJ                # rows per core
W = 64                       # gather window width (classes)
# Static window schedule: classes advance 128/131.072 per column group.
OFF = [int(j * 128 * C / B) for j in range(NJ)]

_PROGRAMS = {}


def _tile_plan():
    return [(1, 2), (2, 1), (4, 30), (2, 1), (1, 2)]


def _iter_tiles():
    row, col = 0, 0
    for g_rpp, cnt in _tile_plan():
        for _ in range(cnt):
            yield row, col, g_rpp
            row += P * g_rpp
            col += g_rpp


def _build_sorted(ncols=C):
    """bf16 stream + windowed gather + batched exp (fast path)."""
    from contextlib import ExitStack

    import concourse.tile as tile
    from concourse import bacc, mybir

    nj = NJ

    nc = bacc.Bacc(
        "TRN2",
        target_bir_lowering=False,
        debug=False,
        enable_asserts=False,
        num_devices=N_CORES,
    )
    x = nc.dram_tensor(
        "x", [ROWS, ncols], mybir.dt.bfloat16, kind="ExternalInput"
    ).ap()
    tfr = nc.dram_tensor("tfr", [P, nj], mybir.dt.bfloat16, kind="ExternalInput").ap()
    cfg = nc.dram_tensor("cfg", [1, 1], mybir.dt.uint32, kind="ExternalInput").ap()
    out = nc.dram_tensor("partial", [1, 1], mybir.dt.float32, kind="ExternalOutput").ap()

    with tile.TileContext(nc) as tc, ExitStack() as ctx:
        stream = ctx.enter_context(tc.tile_pool(name="stream", bufs=3))
        psum = ctx.enter_context(tc.tile_pool(name="psum", bufs=2, space="PSUM"))
        persist = ctx.enter_context(tc.tile_pool(name="persist", bufs=1))
        lreg = ctx.enter_context(nc.vector.register(name="lreg"))

        sums = persist.tile([P, nj], mybir.dt.float32)
        g = persist.tile([P, nj], mybir.dt.float32)
        eg = persist.tile([P, nj], mybir.dt.float32)
        rec = persist.tile([P, nj], mybir.dt.float32)
        prod = persist.tile([P, nj], mybir.dt.float32)
        tf_t = persist.tile([P, nj], mybir.dt.bfloat16)
        cfg_t = persist.tile([1, 1], mybir.dt.uint32)
        # Small loads on the ACT HWDGE queue: keeps the sync queue free so
        # the first stream DMA issues immediately.
        nc.scalar.dma_start(cfg_t[:], cfg)
        nc.scalar.dma_start(tf_t[:], tfr)

        # Per-core window base register (vector engine: used in vector APs).
        nc.vector.load(lreg, cfg_t[:])
        lo = nc.vector.snap(lreg, min_val=0, max_val=C - W)

        warm = persist.tile([P, 1], mybir.dt.float32)
        nc.gpsimd.memset(warm[:], 0.0)
        nc.scalar.activation(warm[:], warm[:], mybir.ActivationFunctionType.Exp)

        # Window-relative class indices 0..W-1, replicated per partition.
        iota_i = persist.tile([P, W], mybir.dt.int32)
        nc.gpsimd.iota(iota_i[:], pattern=[[1, W]], base=0, channel_multiplier=0)
        iota_b = persist.tile([P, W], mybir.dt.bfloat16)
        nc.vector.tensor_copy(iota_b[:], iota_i[:])

        ones = persist.tile([P, 1], mybir.dt.float32)
        nc.vector.memset(ones[:], 1.0)
        zeros_b = persist.tile([P, ncols], mybir.dt.bfloat16)
        nc.vector.memset(zeros_b[:], 0.0)

        def combine(a, b):
            h = slice(a, b)
            nc.scalar.activation(eg[:, h], g[:, h], mybir.ActivationFunctionType.Exp)
            nc.vector.reciprocal(rec[:, h], sums[:, h])
            nc.vector.tensor_mul(prod[:, h], eg[:, h], rec[:, h])

        boundaries = [32, 64, 96, nj - 4]
        done = 0
        pad_done = {}

        from concourse.bass import ds

        for row0, col0, t_rpp in _iter_tiles():
            xt = x[row0 : row0 + P * t_rpp, :].rearrange("(p r) c -> p (r c)", p=P)
            n = t_rpp * ncols
            t = stream.tile(
                [P, n + 2 * W],
                mybir.dt.bfloat16,
                name=f"t{t_rpp}",
                tag=f"t{t_rpp}",
                bufs=3,
            )
            nc.sync.dma_start(t[:, 0:n], xt)
            # Zero the tail pad so last-row windows read finite bf16.  The
            # pool rotates 3 buffers per tag and nothing else writes the pad
            # bytes, so only the first rotation needs the memset.
            if pad_done.get(t_rpp, 0) < 3:
                nc.vector.memset(t[:, n : n + 2 * W], 0.0)
                pad_done[t_rpp] = pad_done.get(t_rpp, 0) + 1
            # First ns rows: exp + per-row sum on ScalarE (accumulator).
            # Mid tiles mostly keep one; every 6th keeps two, landing the
            # engines balanced (measured: scalar 141us vs vector 148us at
            # pure ns=1; each shifted column moves 1.19us off VectorE for
            # +0.59us on ScalarE).
            if t_rpp == 4:
                mid_idx = (col0 - 4) // 4
                ns = 2 if mid_idx % 6 == 2 else 1
            else:
                ns = min(t_rpp, 2)
            for r in range(ns):
                j = col0 + r
                scr0 = psum.tile([P, ncols], mybir.dt.float32, name="scr0")
                nc.scalar.activation(
                    scr0[:],
                    t[:, r * ncols : (r + 1) * ncols],
                    mybir.ActivationFunctionType.Exp,
                    accum_out=sums[:, j : j + 1],
                )
            # Remaining rows: one batched exp into a bf16 scratch; row sums
            # via the DVE tensor-scalar reduce.
            if t_rpp > ns:
                scr = stream.tile(
                    [P, (t_rpp - ns) * ncols],
                    mybir.dt.bfloat16,
                    name=f"s{t_rpp}",
                    tag=f"s{t_rpp}",
                    bufs=2,
                )
                nc.scalar.activation(
                    scr[:], t[:, ns * ncols : n], mybir.ActivationFunctionType.Exp
                )
            for r in range(t_rpp):
                j = col0 + r
                if r >= ns:
                    nc.vector.tensor_reduce(
                        sums[:, j : j + 1],
                        scr[:, (r - ns) * ncols : (r - ns + 1) * ncols],
                        axis=mybir.AxisListType.X,
                        op=mybir.AluOpType.add,
                    )
                # Windowed one-hot gather of the target logit.
                msk = stream.tile([P, W], mybir.dt.bfloat16, name="msk", bufs=2)
                nc.vector.scalar_tensor_tensor(
                    out=msk[:],
                    in0=iota_b[:],
                    scalar=tf_t[:, j : j + 1],
                    in1=t[:, ds(lo + (r * ncols + OFF[j]), W)],
                    op0=mybir.AluOpType.is_equal,
                    op1=mybir.AluOpType.mult,
                    accum_out=g[:, j : j + 1],
                )
            while boundaries and col0 + t_rpp >= boundaries[0]:
                combine(done, boundaries[0])
                done = boundaries.pop(0)

        combine(done, nj)
        pt = persist.tile([P, 1], mybir.dt.float32)
        nc.vector.tensor_reduce(
            pt[:], prod[:], axis=mybir.AxisListType.X, op=mybir.AluOpType.add
        )
        acc = psum.tile([1, 1], mybir.dt.float32, name="acc", bufs=1)
        nc.tensor.matmul(acc[:], lhsT=pt[:], rhs=ones[:], start=True, stop=True)
        res = persist.tile([1, 1], mybir.dt.float32)
        nc.vector.tensor_copy(res[:], acc[:])
        nc.sync.dma_start(out, res[:])

    nc.compile()
    return nc


def _build_fullscan(ncols=C):
    """f32 full-scan fallback (correct for any targets)."""
    from contextlib import ExitStack

    import concourse.tile as tile
    from concourse import bacc, mybir

    nj = NJ

    nc = bacc.Bacc(
        "TRN2",
        target_bir_lowering=False,
        debug=False,
        enable_asserts=False,
        num_devices=N_CORES,
    )
    x = nc.dram_tensor("x", [ROWS, ncols], mybir.dt.float32, kind="ExternalInput").ap()
    tf = nc.dram_tensor("tf", [P, nj], mybir.dt.float32, kind="ExternalInput").ap()
    out = nc.dram_tensor("partial", [1, 1], mybir.dt.float32, kind="ExternalOutput").ap()

    with tile.TileContext(nc) as tc, ExitStack() as ctx:
        stream = ctx.enter_context(tc.tile_pool(name="stream", bufs=3))
        psum = ctx.enter_context(tc.tile_pool(name="psum", bufs=2, space="PSUM"))
        persist = ctx.enter_context(tc.tile_pool(name="persist", bufs=1))

        sums = persist.tile([P, nj], mybir.dt.float32)
        g = persist.tile([P, nj], mybir.dt.float32)
        eg = persist.tile([P, nj], mybir.dt.float32)
        rec = persist.tile([P, nj], mybir.dt.float32)
        prod = persist.tile([P, nj], mybir.dt.float32)
        tf_t = persist.tile([P, nj], mybir.dt.float32)
        nc.scalar.dma_start(tf_t[:], tf)

        warm = persist.tile([P, 1], mybir.dt.float32)
        nc.gpsimd.memset(warm[:], 0.0)
        nc.scalar.activation(warm[:], warm[:], mybir.ActivationFunctionType.Exp)

        iota_i = persist.tile([P, ncols], mybir.dt.int32)
        nc.gpsimd.iota(iota_i[:], pattern=[[1, ncols]], base=0, channel_multiplier=0)
        iota_f = persist.tile([P, ncols], mybir.dt.float32)
        nc.vector.tensor_copy(iota_f[:], iota_i[:])

        ones = persist.tile([P, 1], mybir.dt.float32)
        nc.vector.memset(ones[:], 1.0)

        def combine(a, b):
            h = slice(a, b)
            nc.scalar.activation(eg[:, h], g[:, h], mybir.ActivationFunctionType.Exp)
            nc.vector.reciprocal(rec[:, h], sums[:, h])
            nc.vector.tensor_mul(prod[:, h], eg[:, h], rec[:, h])

        boundaries = [32, 64, 96, nj - 4]
        done = 0

        for row0, col0, t_rpp in _iter_tiles():
            xt = x[row0 : row0 + P * t_rpp, :].rearrange("(p r) c -> p (r c)", p=P)
            t = stream.tile(
                [P, t_rpp * ncols],
                mybir.dt.float32,
                name=f"t{t_rpp}",
                tag=f"t{t_rpp}",
                bufs=3,
            )
            nc.sync.dma_start(t[:], xt)
            for r in range(t_rpp):
                j = col0 + r
                xs = t[:, r * ncols : (r + 1) * ncols]
                scr = psum.tile([P, ncols], mybir.dt.float32, name="scr")
                nc.scalar.activation(
                    scr[:],
                    xs,
                    mybir.ActivationFunctionType.Exp,
                    accum_out=sums[:, j : j + 1],
                )
                msk = stream.tile([P, ncols], mybir.dt.float32, name="msk", bufs=2)
                nc.vector.scalar_tensor_tensor(
                    out=msk[:],
                    in0=iota_f[:],
                    scalar=tf_t[:, j : j + 1],
                    in1=xs,
                    op0=mybir.AluOpType.is_equal,
                    op1=mybir.AluOpType.mult,
                    accum_out=g[:, j : j + 1],
                )
            while boundaries and col0 + t_rpp >= boundaries[0]:
                combine(done, boundaries[0])
                done = boundaries.pop(0)

        combine(done, nj)
        pt = persist.tile([P, 1], mybir.dt.float32)
        nc.vector.tensor_reduce(
            pt[:], prod[:], axis=mybir.AxisListType.X, op=mybir.AluOpType.add
        )
        acc = psum.tile([1, 1], mybir.dt.float32, name="acc", bufs=1)
        nc.tensor.matmul(acc[:], lhsT=pt[:], rhs=ones[:], start=True, stop=True)
        res = persist.tile([1, 1], mybir.dt.float32)
        nc.vector.tensor_copy(res[:], acc[:])
        nc.sync.dma_start(out, res[:])

    nc.compile()
    return nc


def _dev_perm():
    """idx_dev[row0 + p*rpp + r] = (col0 + r) * 128 + p  (per-core local)."""
    idx = np.empty(ROWS, dtype=np.int64)
    for row0, col0, t_rpp in _iter_tiles():
        p = np.arange(P)[:, None]
        r = np.arange(t_rpp)[None, :]
        idx[(row0 + p * t_rpp + r).ravel()] = ((col0 + r) * P + p).ravel()
    return idx


def _plan_windows(tsc):
    """Given a core's ascending-sorted targets, pick the runtime window
    base lo such that [lo+OFF[j], lo+OFF[j]+W) covers sub-tile j's
    targets for all j. Returns lo or None if infeasible."""
    mint = tsc.reshape(NJ, P)[:, 0].astype(np.int64)
    maxt = tsc.reshape(NJ, P)[:, -1].astype(np.int64)
    off = np.asarray(OFF, dtype=np.int64)
    lo_low = int(np.max(maxt - (W - 1) - off))
    lo_high = int(np.min(mint - off))
    if lo_low > lo_high:
        return None
    lo = (lo_low + lo_high) // 2
    return int(np.clip(lo, 0, C - W))


def _run(outputs, targets, trace=False):
    import ml_dtypes

    from concourse import bass_utils

    outputs = np.ascontiguousarray(np.asarray(outputs, dtype=np.float32))
    targets = np.asarray(targets).astype(np.int64)

    # Sort rows by target; shard contiguous sorted ranges per core.
    order = np.argsort(targets, kind="stable")
    dev = _dev_perm()
    plans = []
    ok = True
    for i in range(N_CORES):
        sl = order[i * ROWS : (i + 1) * ROWS]
        tsc = targets[sl]
        lo = _plan_windows(tsc)
        if lo is None:
            ok = False
            break
        plans.append((sl, tsc, lo))

    in_maps = []
    if ok:
        key = "sorted"
        if key not in _PROGRAMS:
            _PROGRAMS[key] = _build_sorted()
        prog = _PROGRAMS[key]
        x16 = outputs.astype(ml_dtypes.bfloat16)
        off = np.asarray(OFF, dtype=np.int64)
        for sl, tsc, lo in plans:
            xd = x16[sl[dev]]
            # tf_rel[p, j] = t - (lo + OFF[j])  in [0, W)
            rel = (tsc.reshape(NJ, P).T - (lo + off)[None, :]).astype(np.float32)
            assert rel.min() >= 0 and rel.max() < W
            in_maps.append(
                {
                    "x": np.ascontiguousarray(xd),
                    "tfr": rel.astype(ml_dtypes.bfloat16),
                    "cfg": np.array([[lo]], dtype=np.uint32),
                }
            )
    else:
        key = "fullscan"
        if key not in _PROGRAMS:
            _PROGRAMS[key] = _build_fullscan()
        prog = _PROGRAMS[key]
        for i in range(N_CORES):
            sl = slice(i * ROWS, (i + 1) * ROWS)
            tfv = np.empty((P, NJ), dtype=np.float32)
            tshard = targets[sl].astype(np.float32)
            for row0, col0, t_rpp in _iter_tiles():
                ridx = (
                    row0
                    + np.arange(P)[:, None] * t_rpp
                    + np.arange(t_rpp)[None, :]
                )
                tfv[:, col0 : col0 + t_rpp] = tshard[ridx]
            in_maps.append({"x": outputs[sl], "tf": tfv})

    kw = {"trace_cores": list(range(N_CORES))} if trace else {}
    results = bass_utils.run_bass_kernel_spmd(
        prog, in_maps, core_ids=list(range(N_CORES)), trace=trace, **kw
    )
    total = sum(float(r["partial"][0, 0]) for r in results.results)
    loss = np.float32(2.0) - np.float32(2.0) * np.float32(total / B)
    return np.asarray(loss, dtype=np.float32), results


def kernel(outputs, targets):
    loss, _ = _run(outputs, targets, trace=False)
    return loss
